# revision 4
# baseline (speedup 1.0000x reference)
"""GPTQ 4-bit quantized linear (column-parallel over 8 NeuronCores), v2.

Computes y = x @ dequant(qweight, scales, zeros).T + bias. qweight holds
byte-packed 4-bit pairs (lo nibble -> even input col, hi nibble -> odd).

v2 strategy (o-tile-major pipeline, per-core out slice 1376 -> 11 tiles of 128):

Host repacks (bit permutation only) the nibbles into per-o-tile byte blocks
byte[p, k*128+o] = nib(i=128k+p, o) | nib(i=2048+128k+p, o) << 4, so chunk k's
lo nibbles are quant group k and hi nibbles group 16+k.

Three per-o-tile unpack paths, chosen to balance the serial devices
(DMA engines / DVE / ACT / Pool under the Tile cost model):
  * A-tiles DMA the bytes pre-widened to u16 with 0x6400 in the high bits:
    the DMA'd tile IS an f16 tensor v = 1024 + lo + 16*hi. One DVE
    tensor_scalar (AND 0x00F0, OR 0xE400, 4x perf mode) builds
    hneg = -(1024 + 16*hi). No converts at all.
  * B-tiles DMA raw u8; ACT converts byte -> f16 (v = lo + 16*hi) and
    converts the DVE-masked hi byte with scale=-1 (hneg = -16*hi).
  * C-tiles DMA raw u8; Pool (gpsimd) converts v; DVE converts hneg via
    tensor_scalar mult -1.
Per chunk k three N=8 matmuls: slot k accumulates v@x_k + hneg@x_k (the hi
and offset parts cancel exactly in fp32 PSUM, leaving the lo-group partial
dot), slot 16+k gets hneg@x_{16+k} = -(16*D_hi [+ 1024*X]).

PSUM d-tiles are [128, 8(b), 33(slot)]; slot 32 is written by a K=33 matmul
szT.T @ XT that folds the zero-point correction AND the bias:
  szT rows g<16: s*z ; rows 16..31: s*(z [+64]) ; row 32: -bias
  XT rows: X[g,b] = sum_{i in g} x[b,i] (on-device ones-matmul), row 32 = 1.
The fix is then one broadcast multiply by sexp (s for lo slots, -s/16 for hi
slots, -1 for slot 32) and one f16 4x-mode reduce over slots straight into
the output staging tile.
"""

import numpy as np

import concourse.bacc as bacc
import concourse.bass_isa as bass_isa
import concourse.mybir as mybir
import concourse.tile as tile
from concourse.bass_utils import run_bass_kernel_spmd

dt = mybir.dt
Alu = mybir.AluOpType
Act = mybir.ActivationFunctionType

B = 8          # batch
I = 4096       # in_features
O = 11008      # out_features
NCORES = 8
OSH = O // NCORES          # 1376 out features per core
OT = 11                    # o-tiles of 128 per core (padded)
OPAD = OT * 128            # 1408
NG = 32                    # quant groups (group size 128)
NK = 16                    # byte chunks per o-tile (2048 packed rows / 128)
NS = NG + 1                # PSUM slots per o-tile (32 groups + corr/bias)

# Per-o-tile unpack path: 'A' u16 DMA + DVE masks, 'B' u8 + ACT converts,
# 'C' u8 + Pool convert (v) / DVE convert (hneg). Interleaved for overlap.
PATHS = "CBACBACBAAA"
# Fix-stage engine split per tile (True -> Pool / gpsimd, else DVE).
FIXMULT_POOL = ()
REDUCE_POOL = ()
# Emission step at which tile t's fix ops are queued: unpack-path latency
# differs (A-tiles' inputs are ready ~2 tiles after their DMA, C ~4, B ~5),
# so later-ready fixes are queued later to avoid FIFO head-of-line blocking.
FIX_DELAY = {"A": 2, "B": 2, "C": 2}

_nc_cache = None


def _build_nc(paths=PATHS, fixmult_pool=FIXMULT_POOL, reduce_pool=REDUCE_POOL,
              warm=0, fix_delay=None):
    if fix_delay is None:
        fix_delay = FIX_DELAY
    na = paths.count("A")
    nb8 = len(paths) - na
    nc = bacc.Bacc("TRN2", target_bir_lowering=False)

    wqA = nc.dram_tensor("wqA", [max(na, 1), 128, NK * 128], dt.float16,
                         kind="ExternalInput")
    wq8 = nc.dram_tensor("wq8", [max(nb8, 1), 128, NK * 128], dt.uint8,
                         kind="ExternalInput")
    xt = nc.dram_tensor("xt", [128, NG * B], dt.float16, kind="ExternalInput")
    sexp = nc.dram_tensor("sexp", [128, OT * B * NS], dt.float16, kind="ExternalInput")
    szT = nc.dram_tensor("szT", [NS, OPAD], dt.float32, kind="ExternalInput")
    out = nc.dram_tensor("out", [OPAD, B], dt.float16, kind="ExternalOutput")
    xrow_dram = nc.dram_tensor("xrow_scratch", [NG, B], dt.float32)

    W = NK * 128  # 2048 free elements per unpacked stream

    with tile.TileContext(nc) as tc:
        with (
            tc.tile_pool(name="const", bufs=1) as constp,
            tc.tile_pool(name="wqap", bufs=6) as wqap,
            tc.tile_pool(name="wq8p", bufs=6) as wq8p,
            tc.tile_pool(name="vp", bufs=6) as vp,
            tc.tile_pool(name="hp", bufs=11) as hp,
            tc.tile_pool(name="fixp", bufs=4) as fixp,
            tc.tile_pool(name="dpsp", bufs=1, space="PSUM") as dpsp,
            tc.tile_pool(name="mpsp", bufs=1, space="PSUM") as mpsp,
        ):
            xt_sb = constp.tile([128, NG * B], dt.float16)
            nc.sync.dma_start(xt_sb[:], xt[:])

            if warm:
                junk_sb = constp.tile([128, 512], dt.float16)
                nc.vector.memset(junk_sb[:], 0.0)
                warm_ps = mpsp.tile([128, 512], dt.float32, tag="warm")
                for w in range(warm):
                    nc.tensor.matmul(
                        warm_ps[:], junk_sb[:, :128], junk_sb[:], start=True,
                        stop=True,
                    )

            # X[g, b] = sum over partitions of each x^T group slice, via a
            # ones-matmul + DRAM bounce into [g, b] layout; row 32 = 1 feeds
            # the -bias szT row through the corr matmul.
            ones_sb = constp.tile([128, 1], dt.float16)
            nc.vector.memset(ones_sb[:], 1.0)
            misc_ps = mpsp.tile([128, NG * B], dt.float32, tag="misc")
            nc.tensor.matmul(
                misc_ps[0:1, :], ones_sb[:], xt_sb[:], start=True, stop=True,
            )
            xrow_sb = constp.tile([1, NG * B], dt.float32)
            nc.vector.tensor_copy(xrow_sb[:], misc_ps[0:1, :])
            XT_sb = constp.tile([NS, B], dt.float32)
            nc.vector.memset(XT_sb[NG:NS], 1.0)

            sexp_sb = constp.tile([128, OT * B * NS], dt.float16)
            szT_sb = constp.tile([NS, OPAD], dt.float32)

            d_ps = [
                dpsp.tile([128, B, NS], dt.float32, name=f"d{i}", tag=f"d{i}")
                for i in range(6)
            ]
            out_sb = constp.tile([128, OT * B], dt.float16)
            out32_sb = constp.tile([128, OT * B], dt.float32)

            a_idx = 0
            b_idx = 0
            fix_q = []

            def emit_tile(t):
                nonlocal a_idx, b_idx
                path = paths[t]
                dv = d_ps[t % 6]

                if path == "A":
                    v_sb = wqap.tile([128, W], dt.float16, tag="wqa")
                    nc.sync.dma_start(v_sb[:], wqA[a_idx])
                    a_idx += 1
                    hneg_sb = hp.tile([128, W], dt.float16, tag="hneg")
                    nc.vector.tensor_scalar(
                        hneg_sb[:].bitcast(dt.uint16), v_sb[:].bitcast(dt.uint16),
                        0x00F0, 0xE400, op0=Alu.bitwise_and, op1=Alu.bitwise_or,
                    )
                else:
                    w8_sb = wq8p.tile([128, W], dt.uint8, tag="w8")
                    nc.sync.dma_start(w8_sb[:], wq8[b_idx])
                    b_idx += 1
                    hi8_sb = hp.tile([128, W], dt.uint8, tag="hi8")
                    nc.vector.tensor_scalar(
                        hi8_sb[:].bitcast(dt.uint16), w8_sb[:].bitcast(dt.uint16),
                        0xF0F0, None, op0=Alu.bitwise_and,
                    )
                    v_sb = vp.tile([128, W], dt.float16, tag="v")
                    hneg_sb = hp.tile([128, W], dt.float16, tag="hneg")
                    if path == "B":
                        nc.scalar.activation(v_sb[:], w8_sb[:], Act.Copy)
                        nc.scalar.activation(hneg_sb[:], hi8_sb[:], Act.Copy,
                                             scale=-1.0)
                    else:
                        nc.gpsimd.tensor_copy(v_sb[:], w8_sb[:])
                        nc.vector.tensor_scalar(
                            hneg_sb[:], hi8_sb[:], -1.0, None, op0=Alu.mult
                        )

                for k in range(NK):
                    wcol = slice(k * 128, (k + 1) * 128)
                    xlo = xt_sb[:, k * B:(k + 1) * B]
                    xhi = xt_sb[:, (NK + k) * B:(NK + k + 1) * B]
                    nc.tensor.matmul(dv[:, :, k], v_sb[:, wcol], xlo,
                                     start=True, stop=False)
                    nc.tensor.matmul(dv[:, :, k], hneg_sb[:, wcol], xlo,
                                     start=False, stop=True)
                    nc.tensor.matmul(dv[:, :, NK + k], hneg_sb[:, wcol], xhi,
                                     start=True, stop=True)

            def emit_fix(t):
                dv = d_ps[t % 6]
                # Correction + bias into slot 32 (K=33 fp32 matmul).
                nc.tensor.matmul(
                    dv[:, :, NG], szT_sb[:, t * 128:(t + 1) * 128], XT_sb[:],
                    start=True, stop=True,
                )

                tmp = fixp.tile([128, B, NS], dt.float32, tag="tmp")
                sx_b = sexp_sb[:, t * B * NS:(t + 1) * B * NS].rearrange(
                    "p (b s) -> p b s", b=B
                )
                nc.vector.tensor_tensor(tmp[:], dv[:], sx_b, Alu.mult)
                nc.vector.tensor_reduce(
                    out32_sb[:, t * B:(t + 1) * B], tmp[:],
                    axis=mybir.AxisListType.X, op=Alu.add,
                )

            sched = {}
            for t in range(OT):
                st = min(t + fix_delay[paths[t]], t + 5)
                sched.setdefault(st, []).append(t)
            last_step = max(sched)
            for step in range(max(OT, last_step + 1)):
                if step < OT:
                    emit_tile(step)
                if step == 1:
                    nc.sync.dma_start(
                        xrow_dram[:].rearrange("g b -> (g b)")[None, :], xrow_sb[:]
                    )
                    nc.sync.dma_start(XT_sb[:NG], xrow_dram[:])
                    nc.sync.dma_start(szT_sb[:], szT[:])
                    nc.sync.dma_start(sexp_sb[:], sexp[:])
                for t in sched.get(step, ()):
                    emit_fix(t)

            # Output in two chunks so the bulk transfer overlaps the tail.
            osplit = 8
            outv = out[:].rearrange("(t p) b -> p t b", p=128)
            osbv = out_sb[:].rearrange("p (t b) -> p t b", b=B)
            nc.vector.tensor_copy(out_sb[:, :osplit * B], out32_sb[:, :osplit * B])
            nc.sync.dma_start(outv[:, :osplit], osbv[:, :osplit])
            nc.vector.tensor_copy(out_sb[:, osplit * B:], out32_sb[:, osplit * B:])
            nc.sync.dma_start(outv[:, osplit:], osbv[:, osplit:])

    nc.compile()
    return nc


def _get_nc():
    global _nc_cache
    if _nc_cache is None:
        _nc_cache = _build_nc()
    return _nc_cache


def _prep_inputs(x, qweight, scales, zeros, bias, paths=PATHS):
    x = np.asarray(x)
    qweight = np.asarray(qweight)
    scales = np.asarray(scales).astype(np.float32)
    zeros = np.asarray(zeros).astype(np.float32)
    bias = np.asarray(bias).astype(np.float32)

    # Unpack nibbles (bit permutation only) and transpose to (I, O).
    qb = qweight.astype(np.uint8)            # low byte; values in [0, 256)
    nib = np.empty((O, I), np.uint8)
    nib[:, 0::2] = qb & 15
    nib[:, 1::2] = qb >> 4
    nibT = np.ascontiguousarray(nib.T)       # (4096, 11008)
    # byte[p, k*128+o] = nib[128k+p, o] | nib[2048+128k+p, o] << 4, per o-tile
    lo_t = nibT[:2048].reshape(NK, 128, O)
    hi_t = nibT[2048:].reshape(NK, 128, O)
    packed = lo_t | (hi_t << 4)              # (NK, 128, O)

    # x^T laid out as [128, g*8+b]
    xt_host = np.ascontiguousarray(
        x.T.reshape(NG, 128, B).transpose(1, 0, 2).reshape(128, NG * B)
    ).astype(np.float16)

    a_tiles = [t for t, p in enumerate(paths) if p == "A"]
    b_tiles = [t for t, p in enumerate(paths) if p != "A"]

    in_maps = []
    for c in range(NCORES):
        sl = slice(c * OSH, (c + 1) * OSH)
        wq_c = np.zeros((OPAD, NK, 128), np.uint8)   # [o, k, p]
        wq_c[:OSH] = packed[:, :, sl].transpose(2, 0, 1)
        # [t, p, k*128+o_local]
        wq_tiles = np.ascontiguousarray(
            wq_c.reshape(OT, 128, NK, 128).transpose(0, 3, 2, 1)
        )
        wqA_c = (wq_tiles[a_tiles].astype(np.uint16) | 0x6400).view(np.float16) \
            if a_tiles else np.zeros((1, 128, NK * 128), np.float16)
        wq8_c = wq_tiles[b_tiles].reshape(len(b_tiles), 128, NK * 128) \
            if b_tiles else np.zeros((1, 128, NK * 128), np.uint8)

        s_pad = np.zeros((OPAD, NG), np.float32)
        s_pad[:OSH] = scales[sl]
        z_pad = np.zeros((OPAD, NG), np.float32)
        z_pad[:OSH] = zeros[sl]
        b_pad = np.zeros((OPAD,), np.float32)
        b_pad[:OSH] = bias[sl]

        # sexp[o_part, t*33+s]: s<16 -> s_g ; 16..31 -> -s_g/16 ; 32 -> -1
        sexp_c = np.empty((OT, 128, NS), np.float32)
        s_t = s_pad.reshape(OT, 128, NG)
        sexp_c[:, :, :NK] = s_t[:, :, :NK]
        sexp_c[:, :, NK:NG] = -s_t[:, :, NK:NG] / 16.0
        sexp_c[:, :, NG] = 1.0
        sexp_h = np.ascontiguousarray(
            np.repeat(sexp_c.transpose(1, 0, 2)[:, :, None, :], B, axis=2)
            .reshape(128, OT * B * NS)
        ).astype(np.float16)

        # szT[s, o]: lo groups s*z ; hi groups s*(z [+64 on A-tiles]) ;
        # row 32: -bias
        szT_c = np.empty((NS, OPAD), np.float32)
        szT_c[:NG] = (s_pad * z_pad).T
        hi_off = np.zeros((OPAD,), np.float32)
        for t in a_tiles:
            hi_off[t * 128:(t + 1) * 128] = 64.0
        szT_c[NK:NG] += (s_pad[:, NK:NG].T * hi_off[None, :])
        szT_c[NG] = -b_pad
        szT_c = -szT_c

        in_maps.append({
            "wqA": np.ascontiguousarray(wqA_c.reshape(-1, 128, NK * 128)),
            "wq8": np.ascontiguousarray(wq8_c),
            "xt": xt_host,
            "sexp": sexp_h,
            "szT": np.ascontiguousarray(szT_c),
        })
    return in_maps


def _gather(results):
    y = np.concatenate([r["out"][:OSH] for r in results], axis=0)  # (11008, 8)
    return np.ascontiguousarray(y.T)                               # (8, 11008) f16


def kernel(x, qweight, scales, zeros, bias, _trace=False):
    nc = _get_nc()
    in_maps = _prep_inputs(x, qweight, scales, zeros, bias)
    res = run_bass_kernel_spmd(
        nc, in_maps, core_ids=list(range(NCORES)), trace=_trace
    )
    out = _gather(res.results)
    if _trace:
        return out, res
    return out
